# revision 31
# baseline (speedup 1.0000x reference)
"""Trainium2 Bass kernel for nn_AdaptiveWaveletBank.

out[b, s, n] = sum_k w_s[k] * signal[b, n - wl_s + k]   (complex w, zero-pad)

Strategy:
  - Data-parallel over batch: 16 rows -> 8 cores x 2 rows.
  - The Morlet-like wavelet w_s decays as exp(-0.5 (k/scale)^2): only the
    first ~6.1*scale taps matter (<1e-8 of peak).  Host truncates.
  - Conv as banded matmuls on the TensorEngine: signal tiled 128-wide on
    partitions (several phase-shifted copies), banded Toeplitz A blocks
    (host-built, fp16) as the moving operand, PSUM fp32 accumulation.
    Scales with few taps use an even/odd half-tile mode (two single
    128-col matmuls sharing one A block); long scales use accumulation
    chains over tile shifts.
  - DVE/ACT copy+cast PSUM->fp16 staging laid out so output DMAs are fully
    contiguous; host reassembles complex64.
"""

import numpy as np

import concourse.bacc as bacc
import concourse.bass as bass
import concourse.mybir as mybir
import concourse.tile as tile
from concourse.bass_utils import run_bass_kernel_spmd

B, L, NSC = 16, 32768, 16
CHUNKS = [(0, 4), (4, 8), (8, 12), (12, 16)]
DUMMIES = 8
NCORES = 8
ROWS = B // NCORES          # rows of the batch per core
NT = L // 128               # 256 signal tiles of 128 samples
PAD = 16                    # leading zero tiles (max tile shift)
NUM_OSC = 6.0
ENV_CUT = 1e-8              # truncate wavelet where envelope < this

F16 = mybir.dt.float16
F32 = mybir.dt.float32


def _scales_and_lengths():
    s = np.exp(np.linspace(np.log(1.0), np.log(32.0), NSC))
    lengths = []
    for sc in s:
        wl = min(int(L * 0.5), int(64 * sc))
        wl = max(wl, 8)
        wl = wl if wl % 2 == 0 else wl + 1
        lengths.append(wl)
    return s, lengths


def _wavelets(sc, wl, cf, bw):
    # float32 arithmetic to mirror the jnp reference
    t = np.arange(wl, dtype=np.float32) / (bw * np.float32(max(float(sc), 0.1)))
    env = np.exp(-0.5 * t * t).astype(np.float32)
    ph = (np.float32(2.0 * np.pi / NUM_OSC) * cf * t).astype(np.float32)
    wr = env * np.cos(ph)
    wi = env * np.sin(ph)
    norm = np.max(np.sqrt(wr * wr + wi * wi)) + np.float32(1e-8)
    return (wr / norm).astype(np.float32), (wi / norm).astype(np.float32), env


def _plan(cf, bw, grans=(64, 32, 8)):
    """Per-scale mode/truncation plan + packed A matrix + phase list.

    eo mode: window base delta (mult of 64/32/8, >= wl, <= wl+64-kcut);
    even half-tile reads sig[128m - delta + j], odd sig[128m - delta+64 + j];
    both share A[j, 2u+c] = w[wl - delta + j - u].
    chain mode: accumulate over 128-tile shifts t with a 0/64 phase pick.
    """
    s_vals, wlens = _scales_and_lengths()
    scales = []
    cols = 0
    phases = [0, 64]            # base phases kept first
    for sc, wl in zip(s_vals, wlens):
        wr, wi, env = _wavelets(sc, wl, cf, bw)
        kcut = int(np.sum(env > ENV_CUT))
        kcut = max(1, min(kcut, wl))
        delta = None
        if kcut <= 64 and wl >= 64:
            for gran in grans:
                d = gran * (-(-wl // gran))
                if d <= wl + 64 - kcut:
                    delta = d
                    break
        if delta is not None:
            sub = []
            for eo in range(2):
                di = delta - 64 * eo
                sg = di % 128
                if sg not in phases:
                    phases.append(sg)
                sub.append((phases.index(sg), di // 128))
            scales.append(dict(wl=wl, wr=wr, wi=wi, kcut=kcut, mode="eo",
                               delta=delta, sub=tuple(sub), col=cols))
            cols += 128
            continue
        best = None
        for ph in (0, 64):
            t_hi = (wl - ph + 127) // 128
            t_lo = -(-(wl - ph - kcut - 126) // 128)
            if t_lo < 0 and ph > 0:
                continue
            t_lo = max(0, t_lo)
            if best is None or t_hi - t_lo < best[1] - best[0]:
                best = (t_lo, t_hi, ph)
        t_lo, t_hi, ph = best
        ts = list(range(t_lo, t_hi + 1))
        scales.append(dict(wl=wl, wr=wr, wi=wi, kcut=kcut, mode="chain",
                           ts=ts, col=cols, ph=ph))
        cols += len(ts) * 256

    amat = np.zeros((128, cols), dtype=np.float16)
    j = np.arange(128)[:, None]
    for sp in scales:
        wl, wr, wi, kcut = sp["wl"], sp["wr"], sp["wi"], sp["kcut"]
        if sp["mode"] == "eo":
            u = np.arange(64)[None, :]
            k = wl - sp["delta"] + j - u
            valid = (k >= 0) & (k < kcut)
            kc = np.clip(k, 0, wl - 1)
            blk = np.zeros((128, 128), dtype=np.float32)
            blk[:, 0::2] = np.where(valid, wr[kc], 0.0)
            blk[:, 1::2] = np.where(valid, wi[kc], 0.0)
            amat[:, sp["col"]:sp["col"] + 128] = blk.astype(np.float16)
            continue
        u = np.arange(128)[None, :]
        for i, t in enumerate(sp["ts"]):
            k = wl - sp["ph"] + j - u - 128 * t
            valid = (k >= 0) & (k < kcut)
            kc = np.clip(k, 0, wl - 1)
            blk = np.zeros((128, 256), dtype=np.float32)
            blk[:, 0::2] = np.where(valid, wr[kc], 0.0)
            blk[:, 1::2] = np.where(valid, wi[kc], 0.0)
            off = sp["col"] + i * 256
            amat[:, off:off + 256] = blk.astype(np.float16)
    return scales, amat, phases


def _make_sig(sig_rows, phases):
    """(ROWS, L) fp32 -> (ROWS, NPH, 128, PAD+NT) fp16 tiled/padded.
    Phase copy sigma: x[i] = sig[i - sigma] (zeros outside)."""
    nph = len(phases)
    st = np.zeros((ROWS, nph, 128, PAD + NT), dtype=np.float16)
    s16 = sig_rows.astype(np.float16)
    for r in range(ROWS):
        for p, sg in enumerate(phases):
            x = np.zeros(L, dtype=np.float16)
            if sg == 0:
                x[:] = s16[r]
            else:
                x[sg:] = s16[r][:L - sg]
            st[r, p, :, PAD:] = x.reshape(NT, 128).T
    return st


def _unit_pairs(grp):
    """Scale pairs per group; group 1 reversed so the kernel tail ends on a
    cheap eo unit."""
    return [(grp * 8 + 2 * i, grp * 8 + 2 * i + 1) for i in range(4)]


def _build_nc(scales, acols, nph):
    """Build + schedule + compile the per-core Bass program."""
    nc = bacc.Bacc("TRN2", target_bir_lowering=False, debug=False,
                   num_devices=NCORES)

    sig_d = nc.dram_tensor("sig", [ROWS, nph, 128, PAD + NT], F16,
                           kind="ExternalInput")
    amat_d = nc.dram_tensor("amat", [128, acols], F16, kind="ExternalInput")
    # out[row, half, c, s, 2u+comp] ; n = half*16384 + c*128 + u
    out_d = nc.dram_tensor("out", [ROWS, 2, 128, NSC, 256], F16,
                           kind="ExternalOutput")

    with tile.TileContext(nc) as tc:
        with tc.tile_pool(name="const", bufs=1) as const_pool, \
             tc.tile_pool(name="ob", bufs=16) as ob_pool, \
             tc.tile_pool(name="ps", bufs=1, space="PSUM") as ps_pool:

            wz = const_pool.tile([128, 512], F16, tag="wz")
            wz2 = const_pool.tile([128, 8], F16, tag="wz2")
            nc.gpsimd.memset(wz[:], 0)

            amat_t = const_pool.tile([128, acols], F16, tag="amat")
            sig_all = const_pool.tile([128, nph * ROWS * (PAD + NT)], F16,
                                      tag="sig")

            def acol(s):
                return scales[s]["col"] if s < NSC else acols

            def amat_dma(s0, s1):
                c0, c1 = acol(s0), acol(s1)
                nc.scalar.dma_start(out=amat_t[:, c0:c1],
                                    in_=amat_d.ap()[:, c0:c1])

            # ACT warm-up early: table load (~2.7us) under the input DMAs
            nc.scalar.copy(wz2[:], wz[:, 0:8])

            def sig_dma(p0, p1):
                if p0 == 0 and p1 == nph:
                    nc.scalar.dma_start(
                        out=sig_all[:]
                            .rearrange("j (g m) -> j g m", m=PAD + NT),
                        in_=sig_d.ap().rearrange("r p j m -> j (r p) m"))
                    return
                sbv = sig_all[:].rearrange("j (r p m) -> j r p m",
                                           r=ROWS, p=nph)
                for r in range(ROWS):
                    nc.scalar.dma_start(
                        out=sbv[:, r, p0:p1, :],
                        in_=sig_d.ap()[r, p0:p1]
                            .rearrange("p j m -> j p m"))

            # single input ring, ordered by consumption
            amat_dma(0, CHUNKS[0][1])
            sig_dma(0, min(2, nph))
            if nph > 2:
                amat_dma(*CHUNKS[1])
                sig_dma(2, nph)
                rest = CHUNKS[2:]
            else:
                rest = CHUNKS[1:]
            for c0, c1 in rest:
                amat_dma(c0, c1)

            def sig_slice(r, p, lo, hi):
                base = (r * nph + p) * (PAD + NT)
                return sig_all[:, base + lo:base + hi]

            # HAM warm-up: dummy matmuls keep the PE busy during the input
            # DMAs so real matmuls run at 2.4 GHz from the start
            dmy = ps_pool.tile([128, 2, 512], F32, tag="ps0")
            for _ in range(DUMMIES):
                nc.tensor.matmul(dmy[:, 0, :], wz[:, 0:128], wz[:],
                                 start=True, stop=True)

            pg = 0
            for grp in range(2):
                for row in range(ROWS):
                    for half in range(2):
                        last_rh = (grp == 1 and row == ROWS - 1 and half == 1)
                        ob = ob_pool.tile([128, 8, 256], F16, tag="ob")
                        for pair, (sA, sB) in enumerate(_unit_pairs(grp)):
                            pg += 1
                            ps = ps_pool.tile([128, 2, 512], F32,
                                              tag=f"ps{pg % 4}")
                            for kk, s in enumerate((sA, sB)):
                                sp = scales[s]
                                if sp["mode"] == "eo":
                                    # even/odd half-tile: n = 128m + 64*eo + u
                                    for eo in range(2):
                                        p, q = sp["sub"][eo]
                                        lo = PAD + 128 * half - q
                                        nc.tensor.matmul(
                                            ps[:, kk,
                                               eo * 128:eo * 128 + 128],
                                            sig_slice(row, p, lo, lo + 128),
                                            amat_t[:, sp["col"]:
                                                   sp["col"] + 128],
                                            start=True, stop=True,
                                        )
                                    continue
                                nts = len(sp["ts"])
                                for i, t in enumerate(sp["ts"]):
                                    lo = PAD + 128 * half - t
                                    nc.tensor.matmul(
                                        ps[:, kk, 0:256],
                                        sig_slice(row, sp["ph"] // 64,
                                                  lo, lo + 128),
                                        amat_t[:, sp["col"] + i * 256:
                                               sp["col"] + (i + 1) * 256],
                                        start=(i == 0),
                                        stop=(i == nts - 1),
                                    )
                            d0 = sA % 8
                            dst = ob[:, d0:d0 + 2, :]
                            src2 = ps[:, :, 0:256]
                            if pair < 2:
                                nc.scalar.copy(dst, src2)
                            else:
                                nc.vector.tensor_copy(dst, src2)
                        if last_rh:
                            for q, eng in ((0, nc.sync), (1, nc.scalar)):
                                s0q = grp * 8 + q * 4
                                eng.dma_start(
                                    out=out_d.ap()[row, half, :,
                                                   s0q:s0q + 4, :]
                                        .rearrange("c s i -> c (s i)"),
                                    in_=ob[:, q * 4:(q + 1) * 4, :]
                                        .rearrange("c s i -> c (s i)"),
                                )
                        else:
                            dma_eng = nc.sync if (row + half) % 2 == 0 \
                                else nc.scalar
                            dma_eng.dma_start(
                                out=out_d.ap()[row, half, :,
                                               grp * 8:(grp + 1) * 8, :]
                                    .rearrange("c s i -> c (s i)"),
                                in_=ob[:].rearrange("c s i -> c (s i)"),
                            )
    nc.compile()
    return nc


_CACHE = {}


def _get_nc(key, scales, acols, nph):
    if key not in _CACHE:
        _CACHE[key] = _build_nc(scales, acols, nph)
    return _CACHE[key]


def _plan_key(scales, phases):
    return tuple((sp["mode"], sp["col"], sp.get("delta", -1),
                  tuple(sp.get("sub", ())), tuple(sp.get("ts", ())),
                  sp.get("ph", -1)) for sp in scales) + tuple(phases) \
        + tuple(CHUNKS) + (DUMMIES,)


GRANS = (64,)


def kernel(signal, scales_log, center_freq_log, bandwidth_log):
    signal = np.asarray(signal, dtype=np.float32)
    cf = np.float32(np.exp(np.float32(np.asarray(center_freq_log))))
    bw = np.float32(np.exp(np.float32(np.asarray(bandwidth_log))))

    scales, amat, phases = _plan(cf, bw, GRANS)
    nc = _get_nc(_plan_key(scales, phases), scales, amat.shape[1],
                 len(phases))

    in_maps = []
    for core in range(NCORES):
        st = _make_sig(signal[core * ROWS:(core + 1) * ROWS], phases)
        in_maps.append({"sig": st, "amat": amat})

    res = run_bass_kernel_spmd(nc, in_maps, core_ids=list(range(NCORES)))

    out = np.empty((B, NSC, L), dtype=np.complex64)
    for core in range(NCORES):
        o = np.asarray(res.results[core]["out"], dtype=np.float32)
        # [row, half, c, s, 2u+comp] -> [row, s, half, c, u, comp]
        o = o.transpose(0, 3, 1, 2, 4).reshape(ROWS, NSC, L, 2)
        out[core * ROWS:(core + 1) * ROWS] = o[..., 0] + 1j * o[..., 1]
    return out


# revision 36
# speedup vs baseline: 1.1401x; 1.1401x over previous
"""Trainium2 Bass kernel for nn_AdaptiveWaveletBank.

out[b, s, n] = sum_k w_s[k] * signal[b, n - wl_s + k]   (complex w, zero-pad)

Strategy:
  - Data-parallel over batch: 16 rows -> 8 cores x 2 rows.
  - The Morlet-like wavelet w_s decays as exp(-0.5 (k/scale)^2): only the
    first ~6.1*scale taps matter (<1e-8 of peak).  Host truncates.
  - Conv as banded matmuls on the TensorEngine: signal tiled 128-wide on
    partitions (several phase-shifted copies), banded Toeplitz A blocks
    (host-built, fp16) as the moving operand, PSUM fp32 accumulation.
    Scales with few taps use an even/odd half-tile mode (two single
    128-col matmuls sharing one A block); long scales use accumulation
    chains over tile shifts.
  - DVE/ACT copy+cast PSUM->fp16 staging laid out so output DMAs are fully
    contiguous; host reassembles complex64.
"""

import numpy as np

import concourse.bacc as bacc
import concourse.bass as bass
import concourse.mybir as mybir
import concourse.tile as tile
from concourse.bass_utils import run_bass_kernel_spmd

B, L, NSC = 16, 32768, 16
CHUNKS = [(0, 4), (4, 8), (8, 12), (12, 16)]
DUMMIES = 8
LAST_SPLIT = 4
NCORES = 8
ROWS = B // NCORES          # rows of the batch per core
NT = L // 128               # 256 signal tiles of 128 samples
PAD = 16                    # leading zero tiles (max tile shift)
NUM_OSC = 6.0
ENV_CUT = 1e-8              # truncate wavelet where envelope < this

F16 = mybir.dt.float16
F32 = mybir.dt.float32


def _scales_and_lengths():
    s = np.exp(np.linspace(np.log(1.0), np.log(32.0), NSC))
    lengths = []
    for sc in s:
        wl = min(int(L * 0.5), int(64 * sc))
        wl = max(wl, 8)
        wl = wl if wl % 2 == 0 else wl + 1
        lengths.append(wl)
    return s, lengths


def _wavelets(sc, wl, cf, bw):
    # float32 arithmetic to mirror the jnp reference
    t = np.arange(wl, dtype=np.float32) / (bw * np.float32(max(float(sc), 0.1)))
    env = np.exp(-0.5 * t * t).astype(np.float32)
    ph = (np.float32(2.0 * np.pi / NUM_OSC) * cf * t).astype(np.float32)
    wr = env * np.cos(ph)
    wi = env * np.sin(ph)
    norm = np.max(np.sqrt(wr * wr + wi * wi)) + np.float32(1e-8)
    return (wr / norm).astype(np.float32), (wi / norm).astype(np.float32), env


def _plan(cf, bw, grans=(64, 32, 8)):
    """Per-scale mode/truncation plan + packed A matrix + phase list.

    eo mode: window base delta (mult of 64/32/8, >= wl, <= wl+64-kcut);
    even half-tile reads sig[128m - delta + j], odd sig[128m - delta+64 + j];
    both share A[j, 2u+c] = w[wl - delta + j - u].
    chain mode: accumulate over 128-tile shifts t with a 0/64 phase pick.
    """
    s_vals, wlens = _scales_and_lengths()
    scales = []
    cols = 0
    phases = [0, 64]            # base phases kept first
    for sc, wl in zip(s_vals, wlens):
        wr, wi, env = _wavelets(sc, wl, cf, bw)
        kcut = int(np.sum(env > ENV_CUT))
        kcut = max(1, min(kcut, wl))
        delta = None
        if kcut <= 64 and wl >= 64:
            for gran in grans:
                d = gran * (-(-wl // gran))
                if d <= wl + 64 - kcut:
                    delta = d
                    break
        if delta is not None:
            sub = []
            for eo in range(2):
                di = delta - 64 * eo
                sg = di % 128
                if sg not in phases:
                    phases.append(sg)
                sub.append((phases.index(sg), di // 128))
            scales.append(dict(wl=wl, wr=wr, wi=wi, kcut=kcut, mode="eo",
                               delta=delta, sub=tuple(sub), col=cols))
            cols += 128
            continue
        best = None
        for ph in (0, 64):
            t_hi = (wl - ph + 127) // 128
            t_lo = -(-(wl - ph - kcut - 126) // 128)
            if t_lo < 0 and ph > 0:
                continue
            t_lo = max(0, t_lo)
            if best is None or t_hi - t_lo < best[1] - best[0]:
                best = (t_lo, t_hi, ph)
        t_lo, t_hi, ph = best
        ts = list(range(t_lo, t_hi + 1))
        # nonzero u-range of each tile-shift block (band is zero outside);
        # consecutive blocks overlap by kcut-1 which also orders them
        # one block is a full-width start=True umbrella (every other block
        # then accumulates into already-written columns); pick the block
        # with the widest native band as umbrella, others stream only
        # their nonzero band
        nat = []
        for t in ts:
            C = wl - ph - 128 * t
            u0 = max(0, min(127, C - kcut + 1))
            u1 = min(127, max(0, C + 127))
            nat.append((u0, u1))
        ui = max(range(len(ts)), key=lambda i: nat[i][1] - nat[i][0])
        ts = [ts[ui]] + ts[:ui] + ts[ui + 1:]
        rng = [(0, 127)] + nat[:ui] + nat[ui + 1:]
        scales.append(dict(wl=wl, wr=wr, wi=wi, kcut=kcut, mode="chain",
                           ts=ts, col=cols, ph=ph, rng=tuple(rng)))
        cols += len(ts) * 256

    amat = np.zeros((128, cols), dtype=np.float16)
    j = np.arange(128)[:, None]
    for sp in scales:
        wl, wr, wi, kcut = sp["wl"], sp["wr"], sp["wi"], sp["kcut"]
        if sp["mode"] == "eo":
            u = np.arange(64)[None, :]
            k = wl - sp["delta"] + j - u
            valid = (k >= 0) & (k < kcut)
            kc = np.clip(k, 0, wl - 1)
            blk = np.zeros((128, 128), dtype=np.float32)
            blk[:, 0::2] = np.where(valid, wr[kc], 0.0)
            blk[:, 1::2] = np.where(valid, wi[kc], 0.0)
            amat[:, sp["col"]:sp["col"] + 128] = blk.astype(np.float16)
            continue
        u = np.arange(128)[None, :]
        for i, t in enumerate(sp["ts"]):
            k = wl - sp["ph"] + j - u - 128 * t
            valid = (k >= 0) & (k < kcut)
            kc = np.clip(k, 0, wl - 1)
            blk = np.zeros((128, 256), dtype=np.float32)
            blk[:, 0::2] = np.where(valid, wr[kc], 0.0)
            blk[:, 1::2] = np.where(valid, wi[kc], 0.0)
            off = sp["col"] + i * 256
            amat[:, off:off + 256] = blk.astype(np.float16)
    return scales, amat, phases


def _make_sig(sig_rows, phases):
    """(ROWS, L) fp32 -> (ROWS, NPH, 128, PAD+NT) fp16 tiled/padded.
    Phase copy sigma: x[i] = sig[i - sigma] (zeros outside)."""
    nph = len(phases)
    st = np.zeros((ROWS, nph, 128, PAD + NT), dtype=np.float16)
    s16 = sig_rows.astype(np.float16)
    for r in range(ROWS):
        for p, sg in enumerate(phases):
            x = np.zeros(L, dtype=np.float16)
            if sg == 0:
                x[:] = s16[r]
            else:
                x[sg:] = s16[r][:L - sg]
            st[r, p, :, PAD:] = x.reshape(NT, 128).T
    return st


def _unit_pairs(grp):
    """Scale pairs per group; group 1 reversed so the kernel tail ends on a
    cheap eo unit."""
    return [(grp * 8 + 2 * i, grp * 8 + 2 * i + 1) for i in range(4)]


def _build_nc(scales, acols, nph):
    """Build + schedule + compile the per-core Bass program."""
    nc = bacc.Bacc("TRN2", target_bir_lowering=False, debug=False,
                   num_devices=NCORES)

    sig_d = nc.dram_tensor("sig", [ROWS, nph, 128, PAD + NT], F16,
                           kind="ExternalInput")
    amat_d = nc.dram_tensor("amat", [128, acols], F16, kind="ExternalInput")
    # out[row, half, c, s, 2u+comp] ; n = half*16384 + c*128 + u
    out_d = nc.dram_tensor("out", [ROWS, 2, 128, NSC, 256], F16,
                           kind="ExternalOutput")

    with tile.TileContext(nc) as tc:
        with tc.tile_pool(name="const", bufs=1) as const_pool, \
             tc.tile_pool(name="ob", bufs=16) as ob_pool, \
             tc.tile_pool(name="ps", bufs=1, space="PSUM") as ps_pool:

            wz = const_pool.tile([128, 512], F16, tag="wz")
            wz2 = const_pool.tile([128, 8], F16, tag="wz2")
            nc.gpsimd.memset(wz[:], 0)

            amat_t = const_pool.tile([128, acols], F16, tag="amat")
            sig_all = const_pool.tile([128, nph * ROWS * (PAD + NT)], F16,
                                      tag="sig")

            def acol(s):
                return scales[s]["col"] if s < NSC else acols

            def amat_dma(s0, s1):
                c0, c1 = acol(s0), acol(s1)
                nc.scalar.dma_start(out=amat_t[:, c0:c1],
                                    in_=amat_d.ap()[:, c0:c1])

            # ACT warm-up early: table load (~2.7us) under the input DMAs
            nc.scalar.copy(wz2[:], wz[:, 0:8])

            def sig_dma(p0, p1):
                if p0 == 0 and p1 == nph:
                    nc.scalar.dma_start(
                        out=sig_all[:]
                            .rearrange("j (g m) -> j g m", m=PAD + NT),
                        in_=sig_d.ap().rearrange("r p j m -> j (r p) m"))
                    return
                sbv = sig_all[:].rearrange("j (r p m) -> j r p m",
                                           r=ROWS, p=nph)
                for r in range(ROWS):
                    nc.scalar.dma_start(
                        out=sbv[:, r, p0:p1, :],
                        in_=sig_d.ap()[r, p0:p1]
                            .rearrange("p j m -> j p m"))

            # single input ring, ordered by consumption
            amat_dma(0, CHUNKS[0][1])
            sig_dma(0, min(2, nph))
            if nph > 2:
                amat_dma(*CHUNKS[1])
                sig_dma(2, nph)
                rest = CHUNKS[2:]
            else:
                rest = CHUNKS[1:]
            for c0, c1 in rest:
                amat_dma(c0, c1)

            def sig_slice(r, p, lo, hi):
                base = (r * nph + p) * (PAD + NT)
                return sig_all[:, base + lo:base + hi]

            # HAM warm-up: dummy matmuls keep the PE busy during the input
            # DMAs so real matmuls run at 2.4 GHz from the start
            dmy = ps_pool.tile([128, 2, 512], F32, tag="ps0")
            for _ in range(DUMMIES):
                nc.tensor.matmul(dmy[:, 0, :], wz[:, 0:128], wz[:],
                                 start=True, stop=True)

            pg = 0
            for grp in range(2):
                for row in range(ROWS):
                    for half in range(2):
                        last_rh = (grp == 1 and row == ROWS - 1 and half == 1)
                        ob = ob_pool.tile([128, 8, 256], F16, tag="ob")
                        for pair, (sA, sB) in enumerate(_unit_pairs(grp)):
                            pg += 1
                            ps = ps_pool.tile([128, 2, 512], F32,
                                              tag=f"ps{pg % 4}")
                            for kk, s in enumerate((sA, sB)):
                                sp = scales[s]
                                if sp["mode"] == "eo":
                                    # even/odd half-tile: n = 128m + 64*eo + u
                                    for eo in range(2):
                                        p, q = sp["sub"][eo]
                                        lo = PAD + 128 * half - q
                                        nc.tensor.matmul(
                                            ps[:, kk,
                                               eo * 128:eo * 128 + 128],
                                            sig_slice(row, p, lo, lo + 128),
                                            amat_t[:, sp["col"]:
                                                   sp["col"] + 128],
                                            start=True, stop=True,
                                        )
                                    continue
                                nts = len(sp["ts"])
                                for i, t in enumerate(sp["ts"]):
                                    lo = PAD + 128 * half - t
                                    u0, u1 = sp["rng"][i]
                                    c0 = sp["col"] + i * 256 + 2 * u0
                                    c1 = sp["col"] + i * 256 + 2 * u1 + 2
                                    nc.tensor.matmul(
                                        ps[:, kk, 2 * u0:2 * u1 + 2],
                                        sig_slice(row, sp["ph"] // 64,
                                                  lo, lo + 128),
                                        amat_t[:, c0:c1],
                                        start=(i == 0),
                                        stop=(i == nts - 1),
                                    )
                            d0 = sA % 8
                            dst = ob[:, d0:d0 + 2, :]
                            src2 = ps[:, :, 0:256]
                            if pair < 2:
                                nc.scalar.copy(dst, src2)
                            else:
                                nc.vector.tensor_copy(dst, src2)
                        if last_rh and LAST_SPLIT == 4:
                            for pr in range(4):
                                eng = nc.sync if pr % 2 == 0 else nc.scalar
                                s0q = grp * 8 + pr * 2
                                eng.dma_start(
                                    out=out_d.ap()[row, half, :,
                                                   s0q:s0q + 2, :]
                                        .rearrange("c s i -> c (s i)"),
                                    in_=ob[:, pr * 2:(pr + 1) * 2, :]
                                        .rearrange("c s i -> c (s i)"),
                                )
                        elif last_rh:
                            for q, eng in ((0, nc.sync), (1, nc.scalar)):
                                s0q = grp * 8 + q * 4
                                eng.dma_start(
                                    out=out_d.ap()[row, half, :,
                                                   s0q:s0q + 4, :]
                                        .rearrange("c s i -> c (s i)"),
                                    in_=ob[:, q * 4:(q + 1) * 4, :]
                                        .rearrange("c s i -> c (s i)"),
                                )
                        else:
                            dma_eng = nc.sync if (row + half) % 2 == 0 \
                                else nc.scalar
                            dma_eng.dma_start(
                                out=out_d.ap()[row, half, :,
                                               grp * 8:(grp + 1) * 8, :]
                                    .rearrange("c s i -> c (s i)"),
                                in_=ob[:].rearrange("c s i -> c (s i)"),
                            )
    nc.compile()
    return nc


_CACHE = {}


def _get_nc(key, scales, acols, nph):
    if key not in _CACHE:
        _CACHE[key] = _build_nc(scales, acols, nph)
    return _CACHE[key]


def _plan_key(scales, phases):
    return tuple((sp["mode"], sp["col"], sp.get("delta", -1),
                  tuple(sp.get("sub", ())), tuple(sp.get("ts", ())),
                  sp.get("ph", -1), tuple(sp.get("rng", ())))
                 for sp in scales) + tuple(phases) \
        + tuple(CHUNKS) + (DUMMIES, LAST_SPLIT)


GRANS = (64,)


def kernel(signal, scales_log, center_freq_log, bandwidth_log):
    signal = np.asarray(signal, dtype=np.float32)
    cf = np.float32(np.exp(np.float32(np.asarray(center_freq_log))))
    bw = np.float32(np.exp(np.float32(np.asarray(bandwidth_log))))

    scales, amat, phases = _plan(cf, bw, GRANS)
    nc = _get_nc(_plan_key(scales, phases), scales, amat.shape[1],
                 len(phases))

    in_maps = []
    for core in range(NCORES):
        st = _make_sig(signal[core * ROWS:(core + 1) * ROWS], phases)
        in_maps.append({"sig": st, "amat": amat})

    res = run_bass_kernel_spmd(nc, in_maps, core_ids=list(range(NCORES)))

    out = np.empty((B, NSC, L), dtype=np.complex64)
    for core in range(NCORES):
        o = np.asarray(res.results[core]["out"], dtype=np.float32)
        # [row, half, c, s, 2u+comp] -> [row, s, half, c, u, comp]
        o = o.transpose(0, 3, 1, 2, 4).reshape(ROWS, NSC, L, 2)
        out[core * ROWS:(core + 1) * ROWS] = o[..., 0] + 1j * o[..., 1]
    return out
